# revision 10
# baseline (speedup 1.0000x reference)
"""DecoderRNN Trainium2 kernel: 63-step LSTM + Luong attention + vocab projection.

Strategy (8 NeuronCores, SPMD):
  - Recurrence: gates computed TRANSPOSED (gatesT chunks [128, 32]) so LSTM
    elementwise runs on 128 partitions and h is produced directly in hT layout.
    W_hhT tiles (bf16) are the stationary operand, h (bf16) the moving one.
    TP=True: the 4096 gate dims are sharded 8 ways (each core owns 128 hidden
    dims x 4 gates); per-step AllGather of the bf16 h-slice [128, 32].
  - Phase 1 (XgT = W_ih x_t + bias, all steps): sharded with the same gate
    split; stored in DRAM, prefetched per step.
  - Phase 3: attention + W_w decoder replicated on every core (b-sharding would
    need core-dependent static APs, which SPMD forbids); the [H, V] vocab
    projection is sharded by vocab: each core computes logits[:, :, slice(4000)].
  - Host side does layout-only prep: transposes, bf16 casts, embedding row
    gather, per-core weight slicing; output is np.concatenate over the V axis.
"""

import numpy as np
import ml_dtypes
from contextlib import ExitStack

import concourse.bass as bass
import concourse.bacc as bacc
import concourse.tile as tile
import concourse.mybir as mybir
from concourse import masks
from concourse.bass_utils import run_bass_kernel_spmd

F32 = mybir.dt.float32
F32R = mybir.dt.float32r
BF16 = mybir.dt.bfloat16
AF = mybir.ActivationFunctionType

B, T, S = 32, 63, 64          # batch, steps (T-1 of the 64), source len
V, E, H = 32000, 512, 1024
G = 4 * H                     # gate dim
P = 128                       # partitions
NCORES = 8
R = T * B                     # 2016 rows, row index r = t*32 + b
VL = V // NCORES              # 4000 vocab slice per core

TP = True                     # shard the recurrence 8-way with per-step AllGather
HDT_IS_F32R = TP              # h/W_hh/scores datapath dtype (f32r under TP)

KH = H // P                   # 8 k-chunks over hidden
KE = E // P                   # 4 k-chunks over embedding
U = 1 if TP else KH           # hidden-dim chunks owned per core (per gate quarter)
CH = 4 * U                    # gate chunks owned per core
NW = 4                        # stage-A row windows
RW = R // NW                  # 504 rows per window
VN = VL // 500                # 8 vocab n-tiles of 500
TGROUPS = [(4 * i, min(4 * i + 4, T)) for i in range((T + 3) // 4)]  # vocab m-tiles


def build_graph():
    nc = bacc.Bacc("TRN2", target_bir_lowering=False, debug=False,
                   num_devices=NCORES)

    def inp(name, shape, dtype):
        return nc.dram_tensor(name, list(shape), dtype, kind="ExternalInput").ap()

    # --- inputs (per-core data may differ, graph is identical) ---
    x_embT = inp("x_embT", [E, R], BF16)           # embedded tgt, transposed
    w_ihT_s = inp("w_ihT_s", [E, CH * P], BF16)    # cols (q,u,p) for owned chunks
    HDT = F32R if HDT_IS_F32R else BF16
    w_hhT_s = inp("w_hhT_s", [H, CH * P], HDT)
    bias_s = inp("bias_s", [P, CH], F32)           # (b_ih+b_hh) per owned chunk
    h0T = inp("h0T", [H, B], HDT)
    c0T_s = inp("c0T_s", [P, U * B], F32)          # c0 slice, cols (u, b)
    enc = inp("enc", [B, S, H], BF16)              # lhsT for context matmul
    encT = inp("encT", [B, H, S], HDT)             # rhs for scores matmul
    w_wT_h = inp("w_wT_h", [H, H], HDT)            # rows 0:H of W_w.T
    w_wT_c = inp("w_wT_c", [H, H], BF16)           # rows H:2H of W_w.T
    b_w_sb = inp("b_w_sb", [P, KH], F32)
    w_outT_s = inp("w_outT_s", [H, VL], BF16)      # per-core vocab slice
    b_out_s = inp("b_out_s", [1, VL], BF16)
    out_s = nc.dram_tensor("out_s", [B, T, VL], F32, kind="ExternalOutput").ap()

    with tile.TileContext(nc) as tc, ExitStack() as ctx:
        pool1 = ctx.enter_context(tc.tile_pool(name="pool1", bufs=1))
        stream = ctx.enter_context(tc.tile_pool(name="stream", bufs=3))
        work = ctx.enter_context(tc.tile_pool(name="work", bufs=2))
        state = ctx.enter_context(tc.tile_pool(name="state", bufs=2))
        ps_gate = ctx.enter_context(tc.tile_pool(name="ps_gate", bufs=1, space="PSUM"))
        ps_mm = ctx.enter_context(tc.tile_pool(name="ps_mm", bufs=2, space="PSUM"))
        dram = ctx.enter_context(tc.tile_pool(name="dram", bufs=1, space="DRAM"))

        # ---------------- resident tiles ----------------
        hall = [pool1.tile([P, R], HDT, name=f"hall{k}") for k in range(KH)]
        dect = [pool1.tile([P, R], BF16, name=f"dect{k}") for k in range(KH)]
        ctxt = [pool1.tile([P, R], BF16, name=f"ctxt{k}") for k in range(KH)]
        whh = pool1.tile([P, KH, CH * P], HDT, name="whh")
        nc.sync.dma_start(whh[:], w_hhT_s.rearrange("(k p) c -> p k c", p=P))
        wih = pool1.tile([P, KE, CH * P], BF16, name="wih")
        nc.sync.dma_start(wih[:], w_ihT_s.rearrange("(k p) c -> p k c", p=P))
        bias_t = pool1.tile([P, CH], F32, name="bias_t")
        nc.sync.dma_start(bias_t[:], bias_s[:])
        bw_t = pool1.tile([P, KH], F32, name="bw_t")
        nc.sync.dma_start(bw_t[:], b_w_sb[:])
        bout_t = pool1.tile([1, VL], BF16, name="bout_t")
        nc.sync.dma_start(bout_t[:], b_out_s[:])
        ones_t = pool1.tile([1, P], BF16, name="ones_t")
        nc.gpsimd.memset(ones_t[:], 1.0)
        h0_t = pool1.tile([P, KH, B], HDT, name="h0_t")
        nc.sync.dma_start(h0_t[:], h0T.rearrange("(k p) b -> p k b", p=P))
        ident = pool1.tile([P, P], BF16, name="ident")
        masks.make_identity(nc, ident[:])

        xg_dram = dram.tile([CH, P, R], F32, name="xg_dram")
        if TP:
            cc_in = [dram.tile([P, B], HDT, name=f"cc_in{i}") for i in range(T)]
            cc_out = [dram.tile([NCORES * P, B], HDT, name=f"cc_out{i}",
                                addr_space="Shared") for i in range(T)]

        # ---------------- stage A: XgT = W_ihT.T @ x_embT + bias ----------------
        _sid = nc.enter_named_scope("stageA", False)[0]
        for n in range(NW):
            xtiles = []
            for k in range(KE):
                xt = stream.tile([P, RW], BF16, name="xa", tag=f"xa{k}", bufs=2)
                nc.sync.dma_start(xt[:], x_embT[k * P:(k + 1) * P, n * RW:(n + 1) * RW])
                xtiles.append(xt)
            for c in range(CH):
                ps = ps_mm.tile([P, RW], F32, name="ps_a", tag="psA")
                for k in range(KE):
                    nc.tensor.matmul(
                        ps[:],
                        lhsT=wih[:, k, c * P:(c + 1) * P],
                        rhs=xtiles[k][:],
                        start=(k == 0), stop=(k == KE - 1))
                xg_sb = work.tile([P, RW], F32, name="xg_sb", tag="xg_sb", bufs=1)
                nc.scalar.activation(xg_sb[:], ps[:], AF.Identity,
                                     bias=bias_t[:, c:c + 1])
                nc.sync.dma_start(xg_dram[c, :, n * RW:(n + 1) * RW], xg_sb[:])

        nc.leave_named_scope("stageA", _sid, False)
        # ---------------- recurrence ----------------
        _sid = nc.enter_named_scope("recur", False)[0]
        c0_sb = pool1.tile([P, U * B], F32, name="c0_sb")
        nc.sync.dma_start(c0_sb[:], c0T_s[:])
        c_prev = None
        for t in range(T):
            # gate matmuls: psum[q] [P, U*B] accumulating over KH hidden chunks
            psg = [ps_gate.tile([P, U * B], F32, name=f"psg{q}", tag=f"psg{q}")
                   for q in range(4)]
            for q in range(4):
                for u in range(U):
                    c_idx = q * U + u
                    for k in range(KH):
                        rhs = (h0_t[:, k, :] if t == 0 else
                               hall[k][:, (t - 1) * B: t * B])
                        nc.tensor.matmul(
                            psg[q][:, u * B:(u + 1) * B],
                            lhsT=whh[:, k, c_idx * P:(c_idx + 1) * P],
                            rhs=rhs,
                            start=(k == 0), stop=(k == KH - 1))
            # Xg prefetch for this step: [CH, P, B] window
            xg_t = stream.tile([P, CH, B], F32, name="xg_t", tag="xg_t")
            nc.sync.dma_start(
                xg_t[:],
                xg_dram[:, :, t * B:(t + 1) * B].rearrange("c p b -> p c b"))
            gq = []
            for q in range(4):
                gs = work.tile([P, U * B], F32, name=f"g{q}", tag=f"g{q}")
                nc.vector.tensor_tensor(
                    out=gs[:], in0=psg[q][:],
                    in1=xg_t[:, q * U:(q + 1) * U, :],
                    op=mybir.AluOpType.add)
                gq.append(gs)
            si = work.tile([P, U * B], F32, name="si", tag="si")
            nc.scalar.activation(si[:], gq[0][:], AF.Sigmoid)
            sf = work.tile([P, U * B], F32, name="sf", tag="sf")
            nc.scalar.activation(sf[:], gq[1][:], AF.Sigmoid)
            tg = work.tile([P, U * B], F32, name="tg", tag="tg")
            nc.scalar.activation(tg[:], gq[2][:], AF.Tanh)
            so = work.tile([P, U * B], F32, name="so", tag="so")
            nc.scalar.activation(so[:], gq[3][:], AF.Sigmoid)
            c_in = (c0_sb if c_prev is None else c_prev)
            c_new = state.tile([P, U * B], F32, name="c_new", tag="c_new")
            t1 = work.tile([P, U * B], F32, name="t1", tag="t1")
            nc.vector.tensor_mul(t1[:], sf[:], c_in[:])
            t2 = work.tile([P, U * B], F32, name="t2", tag="t2")
            nc.vector.tensor_mul(t2[:], si[:], tg[:])
            nc.vector.tensor_add(c_new[:], t1[:], t2[:])
            c_prev = c_new
            tc_t = work.tile([P, U * B], F32, name="tc_t", tag="tc_t")
            nc.scalar.activation(tc_t[:], c_new[:], AF.Tanh)
            if TP:
                h_bf = work.tile([P, B], HDT, name="h_bf", tag="h_bf")
                nc.vector.tensor_mul(h_bf[:], so[:], tc_t[:])
                nc.gpsimd.dma_start(cc_in[t][:], h_bf[:])
                nc.gpsimd.collective_compute(
                    "AllGather", mybir.AluOpType.bypass,
                    replica_groups=[list(range(NCORES))],
                    ins=[cc_in[t].opt()],
                    outs=[cc_out[t].opt()])
                for k in range(KH):
                    nc.sync.dma_start(hall[k][:, t * B:(t + 1) * B],
                                      cc_out[t][k * P:(k + 1) * P, :])
            else:
                for u in range(U):
                    nc.vector.tensor_mul(
                        hall[u][:, t * B:(t + 1) * B],
                        so[:, u * B:(u + 1) * B], tc_t[:, u * B:(u + 1) * B])

        nc.leave_named_scope("recur", _sid, False)
        # ---------------- attention (replicated over all 32 b) ----------------
        _sid = nc.enter_named_scope("attn", False)[0]
        for b in range(B):
            ps_sc = ps_mm.tile([T, S], F32, name="ps_sc", tag="psA")
            for k in range(KH):
                et = stream.tile([P, S], HDT, name="et", tag="et")
                nc.sync.dma_start(et[:], encT[b, k * P:(k + 1) * P, :])
                hs = hall[k].rearrange("p (t b) -> p t b", b=B)
                nc.tensor.matmul(ps_sc[:], lhsT=hs[:, :, b],
                                 rhs=et[:],
                                 start=(k == 0), stop=(k == KH - 1))
            mx = work.tile([T, 1], F32, name="mx", tag="mx")
            nc.vector.tensor_reduce(mx[:], ps_sc[:], axis=mybir.AxisListType.X,
                                    op=mybir.AluOpType.max)
            nmx = work.tile([T, 1], F32, name="nmx", tag="nmx")
            nc.vector.tensor_scalar_mul(nmx[:], mx[:], -1.0)
            probs = work.tile([T, S], F32, name="probs", tag="probs")
            ssum = work.tile([T, 1], F32, name="ssum", tag="ssum")
            nc.scalar.activation(probs[:], ps_sc[:], AF.Exp, bias=nmx[:],
                                 accum_out=ssum[:])
            rec = work.tile([T, 1], F32, name="rec", tag="rec")
            nc.vector.reciprocal(rec[:], ssum[:])
            pn = work.tile([T, S], BF16, name="pn", tag="pn")
            nc.scalar.mul(pn[:], probs[:], rec[:])
            ps_at = ps_mm.tile([S, T], BF16, name="ps_at", tag="psB")
            nc.tensor.transpose(ps_at[:], pn[:], ident[:T, :T])
            attnT = work.tile([S, T], BF16, name="attnT", tag="attnT")
            nc.vector.tensor_copy(attnT[:], ps_at[:])
            for k in range(KH):
                ec = stream.tile([S, P], BF16, name="ec", tag="ec")
                nc.sync.dma_start(ec[:], enc[b, :, k * P:(k + 1) * P])
                ps_cx = ps_mm.tile([P, T], F32, name="ps_cx", tag="psA")
                nc.tensor.matmul(ps_cx[:], lhsT=ec[:],
                                 rhs=attnT[:], start=True, stop=True)
                nc.vector.tensor_copy(
                    ctxt[k].rearrange("p (t b) -> p t b", b=B)[:, :, b], ps_cx[:])

        nc.leave_named_scope("attn", _sid, False)
        # ---------------- decT = tanh(W_wT.T @ [h; ctx] + b_w) ----------------
        _sid = nc.enter_named_scope("decproj", False)[0]
        for mo in range(KH):
            wsh, wsc = [], []
            for k in range(KH):
                wh = stream.tile([P, P], HDT, name="wh", tag=f"wh{k}", bufs=2)
                nc.sync.dma_start(wh[:], w_wT_h[k * P:(k + 1) * P, mo * P:(mo + 1) * P])
                wsh.append(wh)
                wc = stream.tile([P, P], BF16, name="wc", tag=f"wc{k}", bufs=2)
                nc.sync.dma_start(wc[:], w_wT_c[k * P:(k + 1) * P, mo * P:(mo + 1) * P])
                wsc.append(wc)
            for quarter in range(4):
                n0, n1 = quarter * (R // 4), (quarter + 1) * (R // 4)
                ps_d = ps_mm.tile([P, R // 4], F32, name="ps_d", tag="psA")
                for k in range(2 * KH):
                    rhs = (hall[k] if k < KH else ctxt[k - KH])[:, n0:n1]
                    lhsT = wsh[k][:] if k < KH else wsc[k - KH][:]
                    nc.tensor.matmul(ps_d[:], lhsT=lhsT,
                                     rhs=rhs, start=(k == 0), stop=(k == 2 * KH - 1))
                nc.scalar.activation(dect[mo][:, n0:n1], ps_d[:], AF.Tanh,
                                     bias=bw_t[:, mo:mo + 1])

        nc.leave_named_scope("decproj", _sid, False)
        # ---------------- vocab projection (V-sharded) ----------------
        _sid = nc.enter_named_scope("vocab", False)[0]
        for n in range(VN):
            wo_tiles = []
            for k in range(KH):
                wo = stream.tile([P, 500], BF16, name="wo", tag=f"wo{k}", bufs=2)
                nc.sync.dma_start(
                    wo[:], w_outT_s[k * P:(k + 1) * P, n * 500:(n + 1) * 500])
                wo_tiles.append(wo)
            for tg_i, (ta, tb) in enumerate(TGROUPS):
                m0, mw = ta * B, (tb - ta) * B
                ps_v = ps_mm.tile([P, 500], F32, name="ps_v", tag="psB")
                for k in range(KH):
                    nc.tensor.matmul(ps_v[:mw, :],
                                     lhsT=dect[k][:, m0:m0 + mw],
                                     rhs=wo_tiles[k][:],
                                     start=(k == 0), stop=False)
                nc.tensor.matmul(ps_v[:mw, :],
                                 lhsT=ones_t[0:1, :mw],
                                 rhs=bout_t[0:1, n * 500:(n + 1) * 500],
                                 start=False, stop=True)
                o_sb = work.tile([P, 500], F32, name="o_sb", tag="o_sb")
                nc.vector.tensor_copy(o_sb[:mw, :], ps_v[:mw, :])
                nc.sync.dma_start(
                    out_s[:, ta:tb, n * 500:(n + 1) * 500].transpose([1, 0, 2]),
                    o_sb[:mw, :])
        nc.leave_named_scope("vocab", _sid, False)
    nc.compile()
    return nc


_CACHE = {}


def _get_graph():
    if "nc" not in _CACHE:
        _CACHE["nc"] = build_graph()
    return _CACHE["nc"]


def _prep(tgt_input, hidden_state, cell_state, encoder_outputs,
          embedding, W_ih, W_hh, b_ih, b_hh, W_w, b_w, W_out, b_out):
    """Host-side layout prep. Returns per-core input maps."""
    f32 = np.float32
    bf16 = ml_dtypes.bfloat16
    idx = np.asarray(tgt_input)[:, :-1].astype(np.int64)    # [B, T]
    emb = np.asarray(embedding, f32)[idx]                   # [B, T, E]
    x_embT = np.ascontiguousarray(emb.transpose(2, 1, 0).reshape(E, R)).astype(bf16)

    w_ihT = np.asarray(W_ih, f32).T                         # [E, G]
    w_hhT = np.asarray(W_hh, f32).T                         # [H, G]
    bias = (np.asarray(b_ih, f32) + np.asarray(b_hh, f32))  # [G]
    h0T = np.ascontiguousarray(np.asarray(hidden_state, f32)[0].T)
    if not TP:
        h0T = h0T.astype(bf16)
    c0T = np.ascontiguousarray(np.asarray(cell_state, f32)[0].T)  # [H, B]
    enc_b = np.asarray(encoder_outputs, f32).astype(bf16)   # [B, S, H]
    encT_b = np.ascontiguousarray(
        np.asarray(encoder_outputs, f32).transpose(0, 2, 1))
    if not TP:
        encT_b = encT_b.astype(bf16)
    w_wT_full = np.ascontiguousarray(np.asarray(W_w, f32).T)
    w_wT_h = w_wT_full[:H]
    if not TP:
        w_wT_h = w_wT_h.astype(bf16)
    w_wT_c = w_wT_full[H:].astype(bf16)
    b_w_sb = np.ascontiguousarray(np.asarray(b_w, f32).reshape(KH, P).T)
    w_outT = np.asarray(W_out, f32).T                       # [H, V]
    b_out_a = np.asarray(b_out, f32)

    in_maps = []
    for m in range(NCORES):
        # owned gate chunks: for quarter q, hidden chunks u -> global col block
        cols = []
        for q in range(4):
            for u in range(U):
                ch = m if TP else u
                j0 = q * H + ch * P
                cols.append(np.arange(j0, j0 + P))
        cols = np.concatenate(cols)                          # [CH*P]
        wih_s = np.ascontiguousarray(w_ihT[:, cols]).astype(bf16)
        whh_s = np.ascontiguousarray(w_hhT[:, cols])
        if not TP:
            whh_s = whh_s.astype(bf16)
        bias_sb = np.ascontiguousarray(bias[cols].reshape(CH, P).T)
        if TP:
            c0_s = np.ascontiguousarray(c0T[m * P:(m + 1) * P, :])
        else:
            c0_s = np.ascontiguousarray(
                c0T.reshape(KH, P, B).transpose(1, 0, 2).reshape(P, U * B))
        in_maps.append({
            "x_embT": x_embT,
            "w_ihT_s": wih_s,
            "w_hhT_s": whh_s,
            "bias_s": bias_sb,
            "h0T": h0T,
            "c0T_s": c0_s,
            "enc": enc_b,
            "encT": encT_b,
            "w_wT_h": w_wT_h,
            "w_wT_c": w_wT_c,
            "b_w_sb": b_w_sb,
            "w_outT_s": np.ascontiguousarray(
                w_outT[:, m * VL:(m + 1) * VL]).astype(bf16),
            "b_out_s": np.ascontiguousarray(
                b_out_a[m * VL:(m + 1) * VL]).reshape(1, VL).astype(bf16),
        })
    return in_maps


def kernel(**inputs) -> np.ndarray:
    nc = _get_graph()
    in_maps = _prep(**inputs)
    res = run_bass_kernel_spmd(nc, in_maps, list(range(NCORES)))
    outs = [res.results[m]["out_s"] for m in range(NCORES)]
    return np.concatenate(outs, axis=2)



# revision 38
# speedup vs baseline: 1.1897x; 1.1897x over previous
"""DecoderRNN Trainium2 kernel: 63-step LSTM + Luong attention + vocab projection.

Strategy (8 NeuronCores, SPMD), v3 — pipelined single pass, split-bf16 h:
  - Recurrence TP-8: each core owns 128 hidden dims x 4 gates (chunk order
    i,f,o,g). Gate psum [128, 4, 32] accumulates bias (one-hot matmul) +
    W_ih x_t (4 k-tiles/chunk) + W_hh h_{t-1} (8 k-tiles/chunk), all bf16
    (FWL weight loads, 1 cyc/col streams).
  - h is carried as a bf16 (hi, lo) pair -- hi = bf16(h), lo = bf16(h - hi)
    -- restoring ~fp23 accuracy in the h @ W_hh and h @ encT products while
    keeping full-rate bf16 matmuls. Per-step AllGather moves the pair
    ([128, 64] bf16); one DMA scatters cc_out into the hall ring.
  - Attention/decoder/vocab work for a finished block of steps is emitted
    as small "filler" quanta between later recurrence steps, so the PE
    chews on it during each AllGather's ~5-6us latency (also keeps the
    HAM clock-gate warm). Softmax uses exp(x)=(1+t)/(1-t), t=tanh(x/2)
    (x<=max-shifted<=0, numerically safe) so the whole kernel stays on the
    sigmoid/tanh ACT table set -- no 2.7us table reloads mid-pipeline.
  - Block cols are b-major (b,t); vocab m-tiles are fixed 128-col windows
    (FWL) and output DMAs split at batch boundaries. Logits emitted bf16;
    host casts to f32. Vocab V-sharded 8 ways; host concatenates.
"""

import numpy as np
import ml_dtypes
from contextlib import ExitStack

import concourse.bass as bass
import concourse.bacc as bacc
import concourse.tile as tile
import concourse.mybir as mybir
from concourse import masks
from concourse.bass_utils import run_bass_kernel_spmd

F32 = mybir.dt.float32
BF16 = mybir.dt.bfloat16
AF = mybir.ActivationFunctionType
ALU = mybir.AluOpType
AX = mybir.AxisListType

B, T, S = 32, 63, 64          # batch, steps (T-1 of the 64), source len
V, E, H = 32000, 512, 1024
P = 128                       # partitions
NCORES = 8
R = T * B                     # 2016 rows, recurrence col index r = t*32 + b
VL = V // NCORES              # 4000 vocab cols per core
KH = H // P                   # 8 k-chunks over hidden
KE = E // P                   # 4 k-chunks over embedding
NQ = 4                        # gate chunks owned per core (i, f, o, g)
RING = 24                     # hall ring slots
VN = 8                        # vocab n-tiles of 500
VT = VL // VN                 # 500
# decreasing block sizes: block j's attention/dec/vocab filler drains during
# block j+1's AllGather gaps; later blocks shrink to limit the exposed tail.
BLOCK_SIZES = [12, 12, 12, 12, 8, 4, 3]
assert sum(BLOCK_SIZES) == T
BLOCKS = []
_t0 = 0
for _bs in BLOCK_SIZES:
    BLOCKS.append((_t0, _t0 + _bs))
    _t0 += _bs
DEBUG = True


def _dma_segments(m0, mw, bs):
    """Split dect col window [m0, m0+mw) at batch boundaries.

    Returns (b_start, t_off, t_take, rel_row, n_batches) with full-batch
    runs merged, so each segment is a rectangle in (b, t)."""
    raw = []
    r = m0
    while r < m0 + mw:
        b, off = divmod(r, bs)
        take = min(m0 + mw - r, bs - off)
        raw.append((b, off, take, r - m0))
        r += take
    merged = []
    for b, off, take, rel in raw:
        if (merged and off == 0 and take == bs and merged[-1][1] == 0
                and merged[-1][2] == bs and merged[-1][0] + merged[-1][4] == b):
            merged[-1] = merged[-1][:4] + (merged[-1][4] + 1,)
            continue
        merged.append((b, off, take, rel, 1))
    return merged


def build_graph():
    nc = bacc.Bacc("TRN2", target_bir_lowering=False, debug=False,
                   num_devices=NCORES)

    def inp(name, shape, dtype):
        return nc.dram_tensor(name, list(shape), dtype, kind="ExternalInput").ap()

    x_embT = inp("x_embT", [E, R], BF16)            # embedded tgt, (k p) x (t,b)
    w_ihT_s = inp("w_ihT_s", [E, NQ * P], BF16)     # cols (c,p), c in (i,f,o,g)
    w_hhT_s = inp("w_hhT_s", [H, NQ * P], BF16)
    bias_s = inp("bias_s", [P, NQ], F32)            # (b_ih+b_hh) per owned chunk
    h0P = inp("h0P", [H, 2 * B], BF16)              # h0 split pair (hi, lo)
    c0T_s = inp("c0T_s", [P, B], F32)               # own hidden chunk of c0
    enc = inp("enc", [B, S, H], BF16)               # ctx lhsT
    encT = inp("encT", [B, H, S], BF16)             # scores rhs
    w_wT = inp("w_wT", [2 * H, H], BF16)
    b_w_sb = inp("b_w_sb", [P, KH], F32)
    w_outT_s = inp("w_outT_s", [H, VL], BF16)       # per-core vocab slice
    b_out_s = inp("b_out_s", [1, VL], BF16)
    out_s = nc.dram_tensor("out_s", [B, T, VL], BF16, kind="ExternalOutput").ap()
    if DEBUG:
        dbg_h = nc.dram_tensor("dbg_h", [T, P, 2 * B], BF16,
                               kind="ExternalOutput").ap()
        dbg_pn = nc.dram_tensor("dbg_pn", [len(BLOCKS), 16, B * S], BF16,
                                kind="ExternalOutput").ap()
        dbg_ctx = nc.dram_tensor("dbg_ctx", [P, KH, R], BF16,
                                 kind="ExternalOutput").ap()
        dbg_dec = nc.dram_tensor("dbg_dec", [P, KH, R], BF16,
                                 kind="ExternalOutput").ap()
        dbg_g = nc.dram_tensor("dbg_g", [2, P, 6 * NQ * B], F32,
                               kind="ExternalOutput").ap()

    x_embT_r = x_embT.rearrange("(k p) r -> p k r", p=P)

    with tile.TileContext(nc) as tc, ExitStack() as ctx:
        pool1 = ctx.enter_context(tc.tile_pool(name="pool1", bufs=1))
        stream = ctx.enter_context(tc.tile_pool(name="stream", bufs=2))
        work = ctx.enter_context(tc.tile_pool(name="work", bufs=2))
        state = ctx.enter_context(tc.tile_pool(name="state", bufs=2))
        psum = ctx.enter_context(tc.tile_pool(name="psum", bufs=2, space="PSUM"))
        dram = ctx.enter_context(tc.tile_pool(name="dram", bufs=1, space="DRAM"))

        # ---------------- resident tiles (small/critical first) ----------------
        whh = pool1.tile([P, KH, NQ * P], BF16, name="whh")
        nc.sync.dma_start(whh[:], w_hhT_s.rearrange("(k p) c -> p k c", p=P))
        wih = pool1.tile([P, KE, NQ * P], BF16, name="wih")
        nc.sync.dma_start(wih[:], w_ihT_s.rearrange("(k p) c -> p k c", p=P))
        bias_t = pool1.tile([P, NQ], F32, name="bias_t")
        nc.sync.dma_start(bias_t[:], bias_s[:])
        bw_t = pool1.tile([P, KH], F32, name="bw_t")
        nc.sync.dma_start(bw_t[:], b_w_sb[:])
        bout_t = pool1.tile([1, VL], BF16, name="bout_t")
        nc.sync.dma_start(bout_t[:], b_out_s[:])
        ones_t = pool1.tile([1, P], BF16, name="ones_t")
        nc.gpsimd.memset(ones_t[:], 1.0)
        h0_t = pool1.tile([P, KH, 2, B], BF16, name="h0_t")
        nc.sync.dma_start(h0_t[:], h0P.rearrange("(k p) tb -> p k tb", p=P)
                          .rearrange("p k (two b) -> p k two b", two=2))
        c0_sb = pool1.tile([P, B], F32, name="c0_sb")
        nc.sync.dma_start(c0_sb[:], c0T_s[:])
        ident = pool1.tile([P, P], BF16, name="ident")
        masks.make_identity(nc, ident[:])
        # big weights on the scalar queue so they don't delay the first steps
        ww = pool1.tile([P, 2 * KH, H], BF16, name="ww")
        nc.scalar.dma_start(ww[:], w_wT.rearrange("(k p) m -> p k m", p=P))
        wout = pool1.tile([P, KH, VL], BF16, name="wout")
        nc.scalar.dma_start(wout[:], w_outT_s.rearrange("(k p) v -> p k v", p=P))

        # hall ring: h (hi, lo bf16) for the last RING steps
        hall = pool1.tile([P, KH, RING, 2, B], BF16, name="hall")

        cc_in = [dram.tile([P, 2 * B], BF16, name=f"cc_in{i}") for i in range(T)]
        cc_out = [dram.tile([NCORES * P, 2 * B], BF16, name=f"cc_out{i}",
                            addr_space="Shared") for i in range(T)]

        # per-block x_emb tiles, prefetched one block ahead
        xe_tiles = {}
        xg_tiles = {}

        def fetch_xe(bi):
            t0, t1 = BLOCKS[bi]
            xe = stream.tile([P, KE, B * (t1 - t0)], BF16, name="xe",
                             tag=f"xe{bi % 2}", bufs=1)
            nc.sync.dma_start(xe[:], x_embT_r[:, :, t0 * B:t1 * B])
            xe_tiles[bi] = xe

        # Xg = W_ih x + bias for a whole block, emitted as an early filler
        # closure one block ahead of use
        def mk_xg(bi):
            t0, t1 = BLOCKS[bi]
            cols = B * (t1 - t0)
            xg = work.tile([P, NQ, 512], F32, name="xg", tag="xg", bufs=2)
            xg_tiles[bi] = xg
            xe = xe_tiles[bi]

            def emit():
                for c in range(NQ):
                    ps_x = psum.tile([P, 512], F32, name="ps_x", tag="ps_dec",
                                     bufs=1)
                    for k in range(KE):
                        nc.tensor.matmul(
                            ps_x[:, :cols],
                            lhsT=wih[:, k, c * P:(c + 1) * P],
                            rhs=xe[:, k, :],
                            start=(k == 0), stop=(k == KE - 1))
                    nc.scalar.activation(xg[:, c, :cols], ps_x[:, :cols],
                                         AF.Identity, bias=bias_t[:, c:c + 1])
            return emit

        # ---------------- filler emission (attention/dec/vocab per block) ----
        def block_closures(bi):
            t0, t1 = BLOCKS[bi]
            bs = t1 - t0
            cols = B * bs            # block cols, b-major (b, t)
            r0 = t0 % RING
            cls = []

            # per-block tiles (tag-rotated, 2 bufs -> adjacent blocks overlap)
            pn_t = work.tile([16, B, S], BF16, name="pn", tag="pn_blk", bufs=1)
            at_t = work.tile([S, B, 16], BF16, name="at", tag="at_blk", bufs=1)
            ctxb = work.tile([P, KH, cols], BF16, name="ctxb", tag="ctx_blk", bufs=1)
            decb = work.tile([P, KH, cols], BF16, name="decb", tag="dec_blk", bufs=1)

            # --- scores + softmax, one closure per quad of 4 batches ---
            def mk_scores(q):
                def emit():
                    ps_s = psum.tile([16, 4, S], F32, name="ps_s", tag="ps_sc",
                                     bufs=1)
                    et4 = stream.tile([P, 4, KH, S], BF16, name="et4",
                                      tag="et4", bufs=2)
                    nc.gpsimd.dma_start(
                        et4[:], encT[4 * q:4 * q + 4, :, :].rearrange(
                            "b (k p) s -> p b k s", p=P))
                    for bq in range(4):
                        b = q * 4 + bq
                        for k in range(KH):
                            for half in (0, 1):
                                nc.tensor.matmul(
                                    ps_s[:bs, bq, :],
                                    lhsT=hall[:, k, r0:r0 + bs, half, b],
                                    rhs=et4[:, bq, k, :],
                                    start=(k == 0 and half == 0),
                                    stop=(k == KH - 1 and half == 1))
                    # softmax over s via exp(x) = (1+t)/(1-t), t = tanh(x/2)
                    mxn = work.tile([16, 1], F32, name="mxn", tag="mxn")
                    nc.vector.tensor_reduce(mxn[:bs, :], ps_s[:bs, :, :],
                                            axis=AX.XY, op=ALU.max, negate=True)
                    nmx2 = work.tile([16, 1], F32, name="nmx2", tag="nmx2")
                    nc.vector.tensor_scalar_mul(nmx2[:bs, :], mxn[:bs, :], 0.5)
                    tq = work.tile([16, 4, S], F32, name="tq", tag="tq", bufs=1)
                    nc.scalar.activation(tq[:bs, :, :], ps_s[:bs, :, :],
                                         AF.Tanh, bias=nmx2[:bs, :], scale=0.5)
                    un = work.tile([16, 4, S], F32, name="un", tag="un", bufs=1)
                    nc.vector.tensor_scalar_add(un[:bs, :, :], tq[:bs, :, :], 1.0)
                    dn = work.tile([16, 4, S], F32, name="dn", tag="dn", bufs=1)
                    nc.vector.tensor_scalar(dn[:bs, :, :], tq[:bs, :, :],
                                            -1.0, 1.0, ALU.mult, ALU.add)
                    vr = work.tile([16, 4, S], F32, name="vr", tag="vr", bufs=1)
                    nc.vector.reciprocal(vr[:bs, :, :], dn[:bs, :, :])
                    pu = work.tile([16, 4, S], F32, name="pu", tag="pu", bufs=1)
                    nc.vector.tensor_tensor(out=pu[:bs, :, :], in0=un[:bs, :, :],
                                            in1=vr[:bs, :, :], op=ALU.mult)
                    zs = work.tile([16, 4], F32, name="zs", tag="zs")
                    nc.vector.tensor_reduce(zs[:bs, :], pu[:bs, :, :],
                                            axis=AX.X, op=ALU.add)
                    rz = work.tile([16, 4], F32, name="rz", tag="rz")
                    nc.vector.reciprocal(rz[:bs, :], zs[:bs, :])
                    for bq in range(4):
                        b = q * 4 + bq
                        nc.vector.tensor_scalar_mul(
                            pn_t[:bs, b, :], pu[:bs, bq, :],
                            rz[:bs, bq:bq + 1])
                return emit

            # pn_t holds probs [t, b, s]; transpose each b to [s, t]
            def mk_transp(pg):
                def emit():
                    if DEBUG and pg == 0:
                        nc.sync.dma_start(
                            dbg_pn[bi, :, :],
                            pn_t[:].rearrange("t b s -> t (b s)"))
                    for b in range(4 * pg, 4 * pg + 4):
                        ps_t = psum.tile([S, 16], BF16, name="ps_t",
                                         tag="ps_tr", bufs=1)
                        nc.tensor.transpose(
                            ps_t[:, :bs], pn_t[:bs, b, :], ident[:bs, :bs])
                        nc.vector.tensor_copy(at_t[:, b, :bs], ps_t[:, :bs])
                return emit

            # context: per k-chunk, all 32 b into one psum bank, b-major cols
            def mk_ctx(k):
                def emit():
                    eca = stream.tile([S, B, P], BF16, name="eca", tag="eca",
                                      bufs=1)
                    nc.gpsimd.dma_start(
                        eca[:], enc[:, :, k * P:(k + 1) * P].rearrange(
                            "b s h -> s b h"))
                    ps_c = psum.tile([P, 512], F32, name="ps_c", tag="ps_ctx",
                                     bufs=1)
                    for b in range(B):
                        nc.tensor.matmul(
                            ps_c[:, b * bs:(b + 1) * bs],
                            lhsT=eca[:, b, :],
                            rhs=at_t[:, b, :bs],
                            start=True, stop=True)
                    nc.vector.tensor_copy(ctxb[:, k, :], ps_c[:, :cols])
                    if DEBUG:
                        nc.sync.dma_start(dbg_ctx[:, k, t0 * B:t1 * B],
                                          ctxb[:, k, :])
                return emit

            # dec = tanh(W_w^T [h; ctx] + b_w), per output chunk mo
            def mk_dec(mo):
                def emit():
                    ps_d = psum.tile([P, 512], F32, name="ps_d", tag="ps_dec",
                                     bufs=1)
                    for k in range(KH):
                        nc.tensor.matmul(
                            ps_d[:, :cols],
                            lhsT=ww[:, k, mo * P:(mo + 1) * P],
                            rhs=hall[:, k, r0:r0 + bs, 0, :].rearrange(
                                "p t b -> p b t"),
                            start=(k == 0), stop=False)
                    for k in range(KH):
                        nc.tensor.matmul(
                            ps_d[:, :cols],
                            lhsT=ww[:, KH + k, mo * P:(mo + 1) * P],
                            rhs=ctxb[:, k, :],
                            start=False, stop=(k == KH - 1))
                    nc.scalar.activation(decb[:, mo, :], ps_d[:, :cols],
                                         AF.Tanh, bias=bw_t[:, mo:mo + 1])
                    if DEBUG:
                        nc.sync.dma_start(dbg_dec[:, mo, t0 * B:t1 * B],
                                          decb[:, mo, :])
                return emit

            # vocab: per n-tile of 500; m-tiles are 128-col windows (FWL)
            def mk_vocab(n):
                def emit():
                    nm = -(-cols // P)
                    for g in range(nm):
                        m0 = g * P
                        mw = min(P, cols - m0)
                        ps_v = psum.tile([P, VT], F32, name="ps_v", tag="ps_v")
                        for k in range(KH):
                            nc.tensor.matmul(
                                ps_v[:mw, :],
                                lhsT=decb[:, k, m0:m0 + mw],
                                rhs=wout[:, k, n * VT:(n + 1) * VT],
                                start=(k == 0), stop=False)
                        nc.tensor.matmul(
                            ps_v[:mw, :],
                            lhsT=ones_t[0:1, :mw],
                            rhs=bout_t[0:1, n * VT:(n + 1) * VT],
                            start=False, stop=True)
                        o_sb = work.tile([P, VT], BF16, name="o_sb", tag="o_sb")
                        nc.vector.tensor_copy(o_sb[:mw, :], ps_v[:mw, :])
                        for (b0, toff, ttake, rel, nb) in _dma_segments(
                                m0, mw, bs):
                            nc.scalar.dma_start(
                                out_s[b0:b0 + nb, t0 + toff:t0 + toff + ttake,
                                      n * VT:(n + 1) * VT],
                                o_sb[rel:rel + nb * ttake, :])
                return emit

            for q in range(8):
                cls.append(mk_scores(q))
            for pg in range(8):
                cls.append(mk_transp(pg))
            for k in range(KH):
                cls.append(mk_ctx(k))
            for mo in range(KH):
                cls.append(mk_dec(mo))
            for n in range(VN):
                cls.append(mk_vocab(n))
            return cls

        # ---------------- recurrence with interleaved filler ----------------
        fetch_xe(0)
        fetch_xe(1)
        mk_xg(0)()
        mk_xg(1)()
        pending = []
        blocks_done = 0
        cur_blk = 0
        c_prev = c0_sb
        for t in range(T):
            if t >= BLOCKS[cur_blk][1]:
                cur_blk += 1
                if cur_blk + 1 < len(BLOCKS):
                    fetch_xe(cur_blk + 1)
                    pending.insert(0, mk_xg(cur_blk + 1))
            t0b = BLOCKS[cur_blk][0]
            xg = xg_tiles[cur_blk]
            rt = t % RING
            # gates psum [p, (c, b)], c in (i, f, o, g)
            # NB: start=True zeroes the whole 2KB psum bank (pending-zero is
            # bank-granular), so exactly ONE start per step.
            psg = psum.tile([P, NQ, B], F32, name="psg", tag="psg")
            # W_hh (h_hi + h_lo): stalls until AllGather(t-1) has landed
            for c in range(NQ):
                for k in range(KH):
                    for half in (0, 1):
                        rhs = (h0_t[:, k, half, :] if t == 0
                               else hall[:, k, (t - 1) % RING, half, :])
                        nc.tensor.matmul(
                            psg[:, c, :],
                            lhsT=whh[:, k, c * P:(c + 1) * P],
                            rhs=rhs,
                            start=(c == 0 and k == 0 and half == 0),
                            stop=(c == NQ - 1 and k == KH - 1 and half == 1))
            # gates = psum + Xg; sigmoid(i,f,o) one instr, tanh(g)
            gs = work.tile([P, NQ, B], F32, name="gs", tag="gs")
            nc.vector.tensor_tensor(
                out=gs[:], in0=psg[:],
                in1=xg[:, :, (t - t0b) * B:(t - t0b + 1) * B],
                op=ALU.add)
            sfo = work.tile([P, 3, B], F32, name="sfo", tag="sfo")
            nc.scalar.activation(sfo[:], gs[:, 0:3, :], AF.Sigmoid)
            tg = work.tile([P, B], F32, name="tg", tag="tg")
            nc.scalar.activation(tg[:], gs[:, 3, :], AF.Tanh)
            t1_ = work.tile([P, B], F32, name="t1_", tag="t1_")
            nc.vector.tensor_tensor(out=t1_[:], in0=sfo[:, 1, :],
                                    in1=c_prev[:], op=ALU.mult)
            t2_ = work.tile([P, B], F32, name="t2_", tag="t2_")
            nc.vector.tensor_tensor(out=t2_[:], in0=sfo[:, 0, :], in1=tg[:],
                                    op=ALU.mult)
            c_new = state.tile([P, B], F32, name="c_new", tag="c_new")
            nc.vector.tensor_tensor(out=c_new[:], in0=t1_[:], in1=t2_[:],
                                    op=ALU.add)
            c_prev = c_new
            tc_t = work.tile([P, B], F32, name="tc_t", tag="tc_t")
            nc.scalar.activation(tc_t[:], c_new[:], AF.Tanh)
            h_f = work.tile([P, B], F32, name="h_f", tag="h_f")
            nc.vector.tensor_tensor(out=h_f[:], in0=sfo[:, 2, :],
                                    in1=tc_t[:], op=ALU.mult)
            h_pair = work.tile([P, 2, B], BF16, name="h_pair", tag="h_pair")
            nc.vector.tensor_copy(h_pair[:, 0, :], h_f[:])
            nc.vector.tensor_tensor(out=h_pair[:, 1, :], in0=h_f[:],
                                    in1=h_pair[:, 0, :], op=ALU.subtract)
            if DEBUG and t < 2:
                dgt = work.tile([P, 6, NQ, B], F32, name="dgt", tag="dgt")
                nc.vector.tensor_copy(dgt[:, 0, :, :], psg[:])
                nc.vector.tensor_copy(dgt[:, 1, 0:3, :], sfo[:])
                nc.vector.tensor_copy(dgt[:, 1, 3, :], tg[:])
                nc.vector.tensor_copy(dgt[:, 2, 0, :], c_new[:])
                nc.vector.tensor_copy(dgt[:, 2, 1, :], tc_t[:])
                nc.vector.tensor_copy(dgt[:, 2, 2, :], h_f[:])
                nc.sync.dma_start(dbg_g[t, :, :],
                                  dgt[:].rearrange("p a c b -> p (a c b)"))
            # exchange h pair: SBUF -> DRAM -> AllGather -> hall ring
            if DEBUG:
                nc.sync.dma_start(dbg_h[t, :, :],
                                  h_pair[:].rearrange("p two b -> p (two b)"))
            nc.gpsimd.dma_start(cc_in[t][:],
                                h_pair[:].rearrange("p two b -> p (two b)"))
            nc.gpsimd.collective_compute(
                "AllGather", ALU.bypass,
                replica_groups=[list(range(NCORES))],
                ins=[cc_in[t].opt()],
                outs=[cc_out[t].opt()])
            nc.sync.dma_start(
                hall[:, :, rt, :, :],
                cc_out[t].rearrange("(k p) (two b) -> p k two b", p=P, two=2))
            # drain filler into this step's AG gap; pace so each block's
            # closures spread over the steps before the next batch arrives
            if pending:
                nxt = BLOCKS[blocks_done][1] if blocks_done < len(BLOCKS) else T
                quota = -(-len(pending) // max(1, nxt - t))
                for _ in range(quota):
                    if pending:
                        pending.pop(0)()
            if blocks_done < len(BLOCKS) and t + 1 == BLOCKS[blocks_done][1]:
                pending.extend(block_closures(blocks_done))
                blocks_done += 1
        while pending:
            pending.pop(0)()
    nc.compile()
    return nc


_CACHE = {}


def _get_graph():
    if "nc" not in _CACHE:
        _CACHE["nc"] = build_graph()
    return _CACHE["nc"]


def _prep(tgt_input, hidden_state, cell_state, encoder_outputs,
          embedding, W_ih, W_hh, b_ih, b_hh, W_w, b_w, W_out, b_out):
    """Host-side layout prep. Returns per-core input maps."""
    f32 = np.float32
    bf16 = ml_dtypes.bfloat16
    idx = np.asarray(tgt_input)[:, :-1].astype(np.int64)    # [B, T]
    emb = np.asarray(embedding, f32)[idx]                   # [B, T, E]
    x_embT = np.ascontiguousarray(
        emb.transpose(2, 1, 0).reshape(E, R)).astype(bf16)

    w_ihT = np.asarray(W_ih, f32).T                         # [E, 4H]
    w_hhT = np.asarray(W_hh, f32).T                         # [H, 4H]
    bias = (np.asarray(b_ih, f32) + np.asarray(b_hh, f32))  # [4H]
    h0T = np.ascontiguousarray(np.asarray(hidden_state, f32)[0].T)  # [H, B]
    h0_hi = h0T.astype(bf16)
    h0_lo = (h0T - h0_hi.astype(f32)).astype(bf16)
    h0P = np.concatenate([h0_hi, h0_lo], axis=1)            # [H, 2B]
    c0T = np.ascontiguousarray(np.asarray(cell_state, f32)[0].T)  # [H, B]
    enc_b = np.asarray(encoder_outputs, f32).astype(bf16)   # [B, S, H]
    encT_b = np.ascontiguousarray(
        np.asarray(encoder_outputs, f32).transpose(0, 2, 1)).astype(bf16)
    w_wT_full = np.ascontiguousarray(np.asarray(W_w, f32).T).astype(bf16)
    b_w_sb = np.ascontiguousarray(np.asarray(b_w, f32).reshape(KH, P).T)
    w_outT = np.asarray(W_out, f32).T                       # [H, V]
    b_out_a = np.asarray(b_out, f32)

    in_maps = []
    for m in range(NCORES):
        # owned gate cols, chunk order (i, f, o, g); PyTorch gate order
        # along 4H is (i, f, g, o) -> quarters (0, 1, 3, 2)
        cols = np.concatenate([np.arange(q * H + m * P, q * H + (m + 1) * P)
                               for q in (0, 1, 3, 2)])
        in_maps.append({
            "x_embT": x_embT,
            "w_ihT_s": np.ascontiguousarray(w_ihT[:, cols]).astype(bf16),
            "w_hhT_s": np.ascontiguousarray(w_hhT[:, cols]).astype(bf16),
            "bias_s": np.ascontiguousarray(bias[cols].reshape(NQ, P).T),
            "h0P": h0P,
            "c0T_s": np.ascontiguousarray(c0T[m * P:(m + 1) * P, :]),
            "enc": enc_b,
            "encT": encT_b,
            "w_wT": w_wT_full,
            "b_w_sb": b_w_sb,
            "w_outT_s": np.ascontiguousarray(
                w_outT[:, m * VL:(m + 1) * VL]).astype(bf16),
            "b_out_s": np.ascontiguousarray(
                b_out_a[m * VL:(m + 1) * VL]).reshape(1, VL).astype(bf16),
        })
    return in_maps


def kernel(**inputs) -> np.ndarray:
    nc = _get_graph()
    in_maps = _prep(**inputs)
    res = run_bass_kernel_spmd(nc, in_maps, list(range(NCORES)))
    outs = [np.asarray(res.results[m]["out_s"], dtype=np.float32)
            for m in range(NCORES)]
    return np.concatenate(outs, axis=2)


# revision 40
# speedup vs baseline: 1.3423x; 1.1283x over previous
"""DecoderRNN Trainium2 kernel: 63-step LSTM + Luong attention + vocab projection.

Strategy (8 NeuronCores, SPMD), v3 — pipelined single pass, split-bf16 h:
  - Recurrence TP-8: each core owns 128 hidden dims x 4 gates (chunk order
    i,f,o,g). Gate psum [128, 4, 32] accumulates bias (one-hot matmul) +
    W_ih x_t (4 k-tiles/chunk) + W_hh h_{t-1} (8 k-tiles/chunk), all bf16
    (FWL weight loads, 1 cyc/col streams).
  - h is carried as a bf16 (hi, lo) pair -- hi = bf16(h), lo = bf16(h - hi)
    -- restoring ~fp23 accuracy in the h @ W_hh and h @ encT products while
    keeping full-rate bf16 matmuls. Per-step AllGather moves the pair
    ([128, 64] bf16); one DMA scatters cc_out into the hall ring.
  - Attention/decoder/vocab work for a finished block of steps is emitted
    as small "filler" quanta between later recurrence steps, so the PE
    chews on it during each AllGather's ~5-6us latency (also keeps the
    HAM clock-gate warm). Softmax uses exp(x)=(1+t)/(1-t), t=tanh(x/2)
    (x<=max-shifted<=0, numerically safe) so the whole kernel stays on the
    sigmoid/tanh ACT table set -- no 2.7us table reloads mid-pipeline.
  - Block cols are b-major (b,t); vocab m-tiles are fixed 128-col windows
    (FWL) and output DMAs split at batch boundaries. Logits emitted bf16;
    host casts to f32. Vocab V-sharded 8 ways; host concatenates.
"""

import numpy as np
import ml_dtypes
from contextlib import ExitStack

import concourse.bass as bass
import concourse.bacc as bacc
import concourse.tile as tile
import concourse.mybir as mybir
from concourse import masks
from concourse.bass_utils import run_bass_kernel_spmd

F32 = mybir.dt.float32
BF16 = mybir.dt.bfloat16
FP16 = mybir.dt.float16
AF = mybir.ActivationFunctionType
ALU = mybir.AluOpType
AX = mybir.AxisListType

B, T, S = 32, 63, 64          # batch, steps (T-1 of the 64), source len
V, E, H = 32000, 512, 1024
P = 128                       # partitions
NCORES = 8
R = T * B                     # 2016 rows, recurrence col index r = t*32 + b
VL = V // NCORES              # 4000 vocab cols per core
KH = H // P                   # 8 k-chunks over hidden
KE = E // P                   # 4 k-chunks over embedding
NQ = 4                        # gate chunks owned per core (i, f, o, g)
RING = 24                     # hall ring slots
VN = 8                        # vocab n-tiles of 500
VT = VL // VN                 # 500
# decreasing block sizes: block j's attention/dec/vocab filler drains during
# block j+1's AllGather gaps; later blocks shrink to limit the exposed tail.
BLOCK_SIZES = [12, 12, 12, 12, 8, 4, 3]
assert sum(BLOCK_SIZES) == T
BLOCKS = []
_t0 = 0
for _bs in BLOCK_SIZES:
    BLOCKS.append((_t0, _t0 + _bs))
    _t0 += _bs
DEBUG = True


def _dma_segments(m0, mw, bs):
    """Split dect col window [m0, m0+mw) at batch boundaries.

    Returns (b_start, t_off, t_take, rel_row, n_batches) with full-batch
    runs merged, so each segment is a rectangle in (b, t)."""
    raw = []
    r = m0
    while r < m0 + mw:
        b, off = divmod(r, bs)
        take = min(m0 + mw - r, bs - off)
        raw.append((b, off, take, r - m0))
        r += take
    merged = []
    for b, off, take, rel in raw:
        if (merged and off == 0 and take == bs and merged[-1][1] == 0
                and merged[-1][2] == bs and merged[-1][0] + merged[-1][4] == b):
            merged[-1] = merged[-1][:4] + (merged[-1][4] + 1,)
            continue
        merged.append((b, off, take, rel, 1))
    return merged


def build_graph():
    nc = bacc.Bacc("TRN2", target_bir_lowering=False, debug=False,
                   num_devices=NCORES)

    def inp(name, shape, dtype):
        return nc.dram_tensor(name, list(shape), dtype, kind="ExternalInput").ap()

    x_embT = inp("x_embT", [E, R], FP16)            # embedded tgt, (k p) x (t,b)
    w_ihT_s = inp("w_ihT_s", [E, NQ * P], FP16)     # cols (c,p), c in (i,f,o,g)
    w_hhT_s = inp("w_hhT_s", [H, NQ * P], FP16)
    bias_s = inp("bias_s", [P, NQ], F32)            # (b_ih+b_hh) per owned chunk
    h0T = inp("h0T", [H, B], FP16)              # h0 split pair (hi, lo)
    c0T_s = inp("c0T_s", [P, B], F32)               # own hidden chunk of c0
    enc = inp("enc", [B, S, H], FP16)               # ctx lhsT
    encT = inp("encT", [B, H, S], FP16)             # scores rhs
    w_wT = inp("w_wT", [2 * H, H], FP16)
    b_w_sb = inp("b_w_sb", [P, KH], F32)
    w_outT_s = inp("w_outT_s", [H, VL], FP16)       # per-core vocab slice
    b_out_s = inp("b_out_s", [1, VL], FP16)
    out_s = nc.dram_tensor("out_s", [B, T, VL], FP16, kind="ExternalOutput").ap()
    if DEBUG:
        dbg_h = nc.dram_tensor("dbg_h", [T, P, 2 * B], BF16,
                               kind="ExternalOutput").ap()
        dbg_pn = nc.dram_tensor("dbg_pn", [len(BLOCKS), 16, B * S], BF16,
                                kind="ExternalOutput").ap()
        dbg_ctx = nc.dram_tensor("dbg_ctx", [P, KH, R], BF16,
                                 kind="ExternalOutput").ap()
        dbg_dec = nc.dram_tensor("dbg_dec", [P, KH, R], BF16,
                                 kind="ExternalOutput").ap()
        dbg_g = nc.dram_tensor("dbg_g", [2, P, 6 * NQ * B], F32,
                               kind="ExternalOutput").ap()

    x_embT_r = x_embT.rearrange("(k p) r -> p k r", p=P)

    with tile.TileContext(nc) as tc, ExitStack() as ctx:
        pool1 = ctx.enter_context(tc.tile_pool(name="pool1", bufs=1))
        stream = ctx.enter_context(tc.tile_pool(name="stream", bufs=2))
        work = ctx.enter_context(tc.tile_pool(name="work", bufs=2))
        state = ctx.enter_context(tc.tile_pool(name="state", bufs=2))
        psum = ctx.enter_context(tc.tile_pool(name="psum", bufs=2, space="PSUM"))
        dram = ctx.enter_context(tc.tile_pool(name="dram", bufs=1, space="DRAM"))

        # ---------------- resident tiles (small/critical first) ----------------
        whh = pool1.tile([P, KH, NQ * P], FP16, name="whh")
        nc.sync.dma_start(whh[:], w_hhT_s.rearrange("(k p) c -> p k c", p=P))
        wih = pool1.tile([P, KE, NQ * P], FP16, name="wih")
        nc.sync.dma_start(wih[:], w_ihT_s.rearrange("(k p) c -> p k c", p=P))
        bias_t = pool1.tile([P, NQ], F32, name="bias_t")
        nc.sync.dma_start(bias_t[:], bias_s[:])
        bw_t = pool1.tile([P, KH], F32, name="bw_t")
        nc.sync.dma_start(bw_t[:], b_w_sb[:])
        bout_t = pool1.tile([1, VL], FP16, name="bout_t")
        nc.sync.dma_start(bout_t[:], b_out_s[:])
        ones_t = pool1.tile([1, P], FP16, name="ones_t")
        nc.gpsimd.memset(ones_t[:], 1.0)
        h0_t = pool1.tile([P, KH, B], FP16, name="h0_t")
        nc.sync.dma_start(h0_t[:], h0T.rearrange("(k p) b -> p k b", p=P))
        c0_sb = pool1.tile([P, B], F32, name="c0_sb")
        nc.sync.dma_start(c0_sb[:], c0T_s[:])
        ident = pool1.tile([P, P], FP16, name="ident")
        masks.make_identity(nc, ident[:])
        # big weights on the scalar queue so they don't delay the first steps
        ww = pool1.tile([P, 2 * KH, H], FP16, name="ww")
        nc.scalar.dma_start(ww[:], w_wT.rearrange("(k p) m -> p k m", p=P))
        wout = pool1.tile([P, KH, VL], FP16, name="wout")
        nc.scalar.dma_start(wout[:], w_outT_s.rearrange("(k p) v -> p k v", p=P))

        # hall ring: h (fp16) for the last RING steps
        hall = pool1.tile([P, KH, RING, B], FP16, name="hall")

        cc_in = [dram.tile([P, B], FP16, name=f"cc_in{i}") for i in range(T)]
        cc_out = [dram.tile([NCORES * P, B], FP16, name=f"cc_out{i}",
                            addr_space="Shared") for i in range(T)]

        # per-block x_emb tiles, prefetched one block ahead
        xe_tiles = {}
        xg_tiles = {}

        def fetch_xe(bi):
            t0, t1 = BLOCKS[bi]
            xe = stream.tile([P, KE, B * (t1 - t0)], FP16, name="xe",
                             tag=f"xe{bi % 2}", bufs=1)
            nc.sync.dma_start(xe[:], x_embT_r[:, :, t0 * B:t1 * B])
            xe_tiles[bi] = xe

        # Xg = W_ih x + bias for a whole block, emitted as an early filler
        # closure one block ahead of use
        def mk_xg(bi):
            t0, t1 = BLOCKS[bi]
            cols = B * (t1 - t0)
            xg = work.tile([P, NQ, 512], F32, name="xg", tag="xg", bufs=2)
            xg_tiles[bi] = xg
            xe = xe_tiles[bi]

            def emit():
                for c in range(NQ):
                    ps_x = psum.tile([P, 512], F32, name="ps_x", tag="ps_dec",
                                     bufs=1)
                    for k in range(KE):
                        nc.tensor.matmul(
                            ps_x[:, :cols],
                            lhsT=wih[:, k, c * P:(c + 1) * P],
                            rhs=xe[:, k, :],
                            start=(k == 0), stop=(k == KE - 1))
                    nc.scalar.activation(xg[:, c, :cols], ps_x[:, :cols],
                                         AF.Identity, bias=bias_t[:, c:c + 1])
            return emit

        # ---------------- filler emission (attention/dec/vocab per block) ----
        def block_closures(bi):
            t0, t1 = BLOCKS[bi]
            bs = t1 - t0
            cols = B * bs            # block cols, b-major (b, t)
            r0 = t0 % RING
            cls = []

            # per-block tiles (tag-rotated, 2 bufs -> adjacent blocks overlap)
            pn_t = work.tile([16, B, S], FP16, name="pn", tag="pn_blk", bufs=1)
            at_t = work.tile([S, B, 16], FP16, name="at", tag="at_blk", bufs=1)
            ctxb = work.tile([P, KH, cols], FP16, name="ctxb", tag="ctx_blk", bufs=1)
            decb = work.tile([P, KH, cols], FP16, name="decb", tag="dec_blk", bufs=1)

            # --- scores + softmax, one closure per quad of 4 batches ---
            def mk_scores(q):
                def emit():
                    ps_s = psum.tile([16, 4, S], F32, name="ps_s", tag="ps_sc",
                                     bufs=1)
                    et4 = stream.tile([P, 4, KH, S], FP16, name="et4",
                                      tag="et4", bufs=2)
                    nc.sync.dma_start(
                        et4[:], encT[4 * q:4 * q + 4, :, :].rearrange(
                            "b (k p) s -> p b k s", p=P))
                    for bq in range(4):
                        b = q * 4 + bq
                        for k in range(KH):
                            nc.tensor.matmul(
                                ps_s[:bs, bq, :],
                                lhsT=hall[:, k, r0:r0 + bs, b],
                                rhs=et4[:, bq, k, :],
                                start=(k == 0), stop=(k == KH - 1))
                    # softmax over s via exp(x) = (1+t)/(1-t), t = tanh(x/2)
                    mxn = work.tile([16, 1], F32, name="mxn", tag="mxn")
                    nc.vector.tensor_reduce(mxn[:bs, :], ps_s[:bs, :, :],
                                            axis=AX.XY, op=ALU.max, negate=True)
                    nmx2 = work.tile([16, 1], F32, name="nmx2", tag="nmx2")
                    nc.vector.tensor_scalar_mul(nmx2[:bs, :], mxn[:bs, :], 0.5)
                    tq = work.tile([16, 4, S], F32, name="tq", tag="tq", bufs=1)
                    nc.scalar.activation(tq[:bs, :, :], ps_s[:bs, :, :],
                                         AF.Tanh, bias=nmx2[:bs, :], scale=0.5)
                    un = work.tile([16, 4, S], F32, name="un", tag="un", bufs=1)
                    nc.vector.tensor_scalar_add(un[:bs, :, :], tq[:bs, :, :], 1.0)
                    dn = work.tile([16, 4, S], F32, name="dn", tag="dn", bufs=1)
                    nc.vector.tensor_scalar(dn[:bs, :, :], tq[:bs, :, :],
                                            -1.0, 1.0, ALU.mult, ALU.add)
                    vr = work.tile([16, 4, S], F32, name="vr", tag="vr", bufs=1)
                    nc.vector.reciprocal(vr[:bs, :, :], dn[:bs, :, :])
                    pu = work.tile([16, 4, S], F32, name="pu", tag="pu", bufs=1)
                    nc.vector.tensor_tensor(out=pu[:bs, :, :], in0=un[:bs, :, :],
                                            in1=vr[:bs, :, :], op=ALU.mult)
                    zs = work.tile([16, 4], F32, name="zs", tag="zs")
                    nc.vector.tensor_reduce(zs[:bs, :], pu[:bs, :, :],
                                            axis=AX.X, op=ALU.add)
                    rz = work.tile([16, 4], F32, name="rz", tag="rz")
                    nc.vector.reciprocal(rz[:bs, :], zs[:bs, :])
                    for bq in range(4):
                        b = q * 4 + bq
                        nc.vector.tensor_scalar_mul(
                            pn_t[:bs, b, :], pu[:bs, bq, :],
                            rz[:bs, bq:bq + 1])
                return emit

            # pn_t holds probs [t, b, s]; transpose each b to [s, t]
            def mk_transp(pg):
                def emit():
                    if DEBUG and pg == 0:
                        nc.sync.dma_start(
                            dbg_pn[bi, :, :],
                            pn_t[:].rearrange("t b s -> t (b s)"))
                    for b in range(4 * pg, 4 * pg + 4):
                        ps_t = psum.tile([S, 16], FP16, name="ps_t",
                                         tag="ps_tr", bufs=1)
                        nc.tensor.transpose(
                            ps_t[:, :bs], pn_t[:bs, b, :], ident[:bs, :bs])
                        nc.vector.tensor_copy(at_t[:, b, :bs], ps_t[:, :bs])
                return emit

            # context: per k-chunk, all 32 b into one psum bank, b-major cols
            def mk_ctx(k):
                def emit():
                    eca = stream.tile([S, B, P], FP16, name="eca", tag="eca",
                                      bufs=1)
                    nc.sync.dma_start(
                        eca[:], enc[:, :, k * P:(k + 1) * P].rearrange(
                            "b s h -> s b h"))
                    ps_c = psum.tile([P, 512], F32, name="ps_c", tag="ps_ctx",
                                     bufs=1)
                    for b in range(B):
                        nc.tensor.matmul(
                            ps_c[:, b * bs:(b + 1) * bs],
                            lhsT=eca[:, b, :],
                            rhs=at_t[:, b, :bs],
                            start=True, stop=True)
                    nc.vector.tensor_copy(ctxb[:, k, :], ps_c[:, :cols])
                    if DEBUG:
                        nc.sync.dma_start(dbg_ctx[:, k, t0 * B:t1 * B],
                                          ctxb[:, k, :])
                return emit

            # dec = tanh(W_w^T [h; ctx] + b_w), per output chunk mo
            def mk_dec(mo):
                def emit():
                    ps_d = psum.tile([P, 512], F32, name="ps_d", tag="ps_dec",
                                     bufs=1)
                    for k in range(KH):
                        nc.tensor.matmul(
                            ps_d[:, :cols],
                            lhsT=ww[:, k, mo * P:(mo + 1) * P],
                            rhs=hall[:, k, r0:r0 + bs, :].rearrange(
                                "p t b -> p b t"),
                            start=(k == 0), stop=False)
                    for k in range(KH):
                        nc.tensor.matmul(
                            ps_d[:, :cols],
                            lhsT=ww[:, KH + k, mo * P:(mo + 1) * P],
                            rhs=ctxb[:, k, :],
                            start=False, stop=(k == KH - 1))
                    nc.scalar.activation(decb[:, mo, :], ps_d[:, :cols],
                                         AF.Tanh, bias=bw_t[:, mo:mo + 1])
                    if DEBUG:
                        nc.sync.dma_start(dbg_dec[:, mo, t0 * B:t1 * B],
                                          decb[:, mo, :])
                return emit

            # vocab: per n-tile of 500; m-tiles are 128-col windows (FWL)
            def mk_vocab(n):
                def emit():
                    nm = -(-cols // P)
                    for g in range(nm):
                        m0 = g * P
                        mw = min(P, cols - m0)
                        ps_v = psum.tile([P, VT], F32, name="ps_v", tag="ps_v")
                        for k in range(KH):
                            nc.tensor.matmul(
                                ps_v[:mw, :],
                                lhsT=decb[:, k, m0:m0 + mw],
                                rhs=wout[:, k, n * VT:(n + 1) * VT],
                                start=(k == 0), stop=False)
                        nc.tensor.matmul(
                            ps_v[:mw, :],
                            lhsT=ones_t[0:1, :mw],
                            rhs=bout_t[0:1, n * VT:(n + 1) * VT],
                            start=False, stop=True)
                        o_sb = work.tile([P, VT], FP16, name="o_sb", tag="o_sb")
                        nc.vector.tensor_copy(o_sb[:mw, :], ps_v[:mw, :])
                        for (b0, toff, ttake, rel, nb) in _dma_segments(
                                m0, mw, bs):
                            nc.scalar.dma_start(
                                out_s[b0:b0 + nb, t0 + toff:t0 + toff + ttake,
                                      n * VT:(n + 1) * VT],
                                o_sb[rel:rel + nb * ttake, :])
                return emit

            for q in range(8):
                cls.append(mk_scores(q))
            for pg in range(8):
                cls.append(mk_transp(pg))
            for k in range(KH):
                cls.append(mk_ctx(k))
            for mo in range(KH):
                cls.append(mk_dec(mo))
            for n in range(VN):
                cls.append(mk_vocab(n))
            return cls

        # ---------------- recurrence with interleaved filler ----------------
        fetch_xe(0)
        fetch_xe(1)
        mk_xg(0)()
        mk_xg(1)()
        pending = []
        blocks_done = 0
        cur_blk = 0
        c_prev = c0_sb
        for t in range(T):
            if t >= BLOCKS[cur_blk][1]:
                cur_blk += 1
                if cur_blk + 1 < len(BLOCKS):
                    fetch_xe(cur_blk + 1)
                    pending.insert(0, mk_xg(cur_blk + 1))
            t0b = BLOCKS[cur_blk][0]
            xg = xg_tiles[cur_blk]
            rt = t % RING
            # gates psum [p, (c, b)], c in (i, f, o, g)
            # NB: start=True zeroes the whole 2KB psum bank (pending-zero is
            # bank-granular), so exactly ONE start per step.
            psg = psum.tile([P, NQ, B], F32, name="psg", tag="psg", bufs=1)
            # W_hh h_{t-1}: stalls until AllGather(t-1) has landed
            for c in range(NQ):
                for k in range(KH):
                    rhs = (h0_t[:, k, :] if t == 0
                           else hall[:, k, (t - 1) % RING, :])
                    nc.tensor.matmul(
                        psg[:, c, :],
                        lhsT=whh[:, k, c * P:(c + 1) * P],
                        rhs=rhs,
                        start=(c == 0 and k == 0),
                        stop=(c == NQ - 1 and k == KH - 1))
            # gates = psum + Xg, landed in PSUM so the ACT reads dodge the
            # SBUF-source errata; sigmoid(i,f,o) one instr, tanh(g)
            gs = psum.tile([P, NQ, B], F32, name="gs", tag="gs", bufs=1)
            nc.vector.tensor_tensor(
                out=gs[:], in0=psg[:],
                in1=xg[:, :, (t - t0b) * B:(t - t0b + 1) * B],
                op=ALU.add)
            sfo = work.tile([P, 3, B], F32, name="sfo", tag="sfo")
            nc.scalar.activation(sfo[:], gs[:, 0:3, :], AF.Sigmoid)
            tg = work.tile([P, B], F32, name="tg", tag="tg")
            nc.scalar.activation(tg[:], gs[:, 3, :], AF.Tanh)
            t1_ = work.tile([P, B], F32, name="t1_", tag="t1_")
            nc.vector.tensor_tensor(out=t1_[:], in0=sfo[:, 1, :],
                                    in1=c_prev[:], op=ALU.mult)
            t2_ = work.tile([P, B], F32, name="t2_", tag="t2_")
            nc.vector.tensor_tensor(out=t2_[:], in0=sfo[:, 0, :], in1=tg[:],
                                    op=ALU.mult)
            c_new = state.tile([P, B], F32, name="c_new", tag="c_new")
            nc.vector.tensor_tensor(out=c_new[:], in0=t1_[:], in1=t2_[:],
                                    op=ALU.add)
            c_prev = c_new
            tc_t = work.tile([P, B], F32, name="tc_t", tag="tc_t")
            nc.scalar.activation(tc_t[:], c_new[:], AF.Tanh)
            h_f = work.tile([P, B], FP16, name="h_f", tag="h_f")
            nc.vector.tensor_tensor(out=h_f[:], in0=sfo[:, 2, :],
                                    in1=tc_t[:], op=ALU.mult)
            # exchange h pair: SBUF -> DRAM -> AllGather -> hall ring
            nc.gpsimd.dma_start(cc_in[t][:], h_f[:])
            nc.gpsimd.collective_compute(
                "AllGather", ALU.bypass,
                replica_groups=[list(range(NCORES))],
                ins=[cc_in[t].opt()],
                outs=[cc_out[t].opt()])
            nc.sync.dma_start(
                hall[:, :, rt, :],
                cc_out[t].rearrange("(k p) b -> p k b", p=P))
            # drain filler into this step's AG gap; pace so each block's
            # closures spread over the steps before the next batch arrives
            if pending:
                nxt = BLOCKS[blocks_done][1] if blocks_done < len(BLOCKS) else T
                quota = -(-len(pending) // max(1, nxt - t))
                for _ in range(quota):
                    if pending:
                        pending.pop(0)()
            if blocks_done < len(BLOCKS) and t + 1 == BLOCKS[blocks_done][1]:
                pending.extend(block_closures(blocks_done))
                blocks_done += 1
        while pending:
            pending.pop(0)()
    nc.compile()
    return nc


_CACHE = {}


def _get_graph():
    if "nc" not in _CACHE:
        _CACHE["nc"] = build_graph()
    return _CACHE["nc"]


def _prep(tgt_input, hidden_state, cell_state, encoder_outputs,
          embedding, W_ih, W_hh, b_ih, b_hh, W_w, b_w, W_out, b_out):
    """Host-side layout prep. Returns per-core input maps."""
    f32 = np.float32
    bf16 = np.float16
    idx = np.asarray(tgt_input)[:, :-1].astype(np.int64)    # [B, T]
    emb = np.asarray(embedding, f32)[idx]                   # [B, T, E]
    x_embT = np.ascontiguousarray(
        emb.transpose(2, 1, 0).reshape(E, R)).astype(bf16)

    w_ihT = np.asarray(W_ih, f32).T                         # [E, 4H]
    w_hhT = np.asarray(W_hh, f32).T                         # [H, 4H]
    bias = (np.asarray(b_ih, f32) + np.asarray(b_hh, f32))  # [4H]
    h0T_a = np.ascontiguousarray(
        np.asarray(hidden_state, f32)[0].T).astype(bf16)    # [H, B]
    c0T = np.ascontiguousarray(np.asarray(cell_state, f32)[0].T)  # [H, B]
    enc_b = np.asarray(encoder_outputs, f32).astype(bf16)   # [B, S, H]
    encT_b = np.ascontiguousarray(
        np.asarray(encoder_outputs, f32).transpose(0, 2, 1)).astype(bf16)
    w_wT_full = np.ascontiguousarray(np.asarray(W_w, f32).T).astype(bf16)
    b_w_sb = np.ascontiguousarray(np.asarray(b_w, f32).reshape(KH, P).T)
    w_outT = np.asarray(W_out, f32).T                       # [H, V]
    b_out_a = np.asarray(b_out, f32)

    in_maps = []
    for m in range(NCORES):
        # owned gate cols, chunk order (i, f, o, g); PyTorch gate order
        # along 4H is (i, f, g, o) -> quarters (0, 1, 3, 2)
        cols = np.concatenate([np.arange(q * H + m * P, q * H + (m + 1) * P)
                               for q in (0, 1, 3, 2)])
        in_maps.append({
            "x_embT": x_embT,
            "w_ihT_s": np.ascontiguousarray(w_ihT[:, cols]).astype(bf16),
            "w_hhT_s": np.ascontiguousarray(w_hhT[:, cols]).astype(bf16),
            "bias_s": np.ascontiguousarray(bias[cols].reshape(NQ, P).T),
            "h0T": h0T_a,
            "c0T_s": np.ascontiguousarray(c0T[m * P:(m + 1) * P, :]),
            "enc": enc_b,
            "encT": encT_b,
            "w_wT": w_wT_full,
            "b_w_sb": b_w_sb,
            "w_outT_s": np.ascontiguousarray(
                w_outT[:, m * VL:(m + 1) * VL]).astype(bf16),
            "b_out_s": np.ascontiguousarray(
                b_out_a[m * VL:(m + 1) * VL]).reshape(1, VL).astype(bf16),
        })
    return in_maps


def kernel(**inputs) -> np.ndarray:
    nc = _get_graph()
    in_maps = _prep(**inputs)
    res = run_bass_kernel_spmd(nc, in_maps, list(range(NCORES)))
    outs = [np.asarray(res.results[m]["out_s"], dtype=np.float32)
            for m in range(NCORES)]
    return np.concatenate(outs, axis=2)
